# revision 111
# baseline (speedup 1.0000x reference)
"""Trainium2 Bass kernel for nn_CapsuleLayer (dynamic routing capsule layer).

Sharding: data-parallel on batch B=64 across 8 NeuronCores (8 per core).
Per core: 1152 positions (b, h*w), tiled 9 x 128 positions on SBUF partitions.

Votes-free routing (recompute preactivations on the PE each iteration):
  pre[pos,o,at] = sum_{i,a} (route[pos,i,o] * x[pos,i,a]) * w[i,a,o,at]

Distances in the TRANSPOSED domain so the PE does both contractions:
  gT[(a,il), (o,pos)] = sum_at w2T[(o,at),(a,il)] * actT[(o,at), pos]
  hT = xT * gT                  (the only vector-engine product)
  dist[pos, (o,i)]  = sum_a hT  (PE matmul against a 0/1 selection matrix)
The squash scale scl is folded into act before transposing, so dist comes
out scaled and feeds logits directly.

Engine notes (cost model): DVE TensorCopy runs 4x (0.26 ns/el) -> route
replication there; TensorTensor is 2x max; ACT is 0.833 ns/el + ~185 fixed
(fuse bias/square into PSUM evacs); Pool ~2 ns/el + 95 launch; PE matmul
cost = out free size only (Ldweights free).
"""

import numpy as np

B, I, A, H, W = 64, 32, 8, 12, 12
HW = H * W                     # 144
O, AT = 10, 16
OAT = O * AT                   # 160
NCORES = 8
BL = B // NCORES               # 8 local batch
NPOS = BL * HW                 # 1152 positions per core
P = 128
NT = NPOS // P                 # 9 tiles
IA = I * A                     # 256
NUM_ROUTING = 3
BIAS_CONST = 0.1               # module bias init (verified at runtime)

_BUILD_CACHE = {}


def _split_multiwait_instructions(nc):
    """This walrus build accepts only ONE sync-wait per instruction.
    Hoist extra waits onto injected single-wait NoOps on the same engine,
    placed immediately before the instruction."""
    from concourse import mybir

    k = 0
    for f in nc.m.functions:
        for b in f.blocks:
            out = []
            changed = False
            for ins in b.instructions:
                si = ins.sync_info
                if si is not None and len(si.on_wait) > 1:
                    waits = list(si.on_wait)
                    for w in waits[:-1]:
                        k += 1
                        out.append(
                            mybir.InstNoOp(
                                name=f"mwsplit-{k}",
                                engine=ins.engine,
                                sync_info=mybir.SyncInfo(
                                    on_wait=[w], on_update=[]
                                ),
                                bass_nofuse=True,
                            )
                        )
                    ins.sync_info = mybir.SyncInfo(
                        on_wait=[waits[-1]], on_update=list(si.on_update)
                    )
                    changed = True
                out.append(ins)
            if changed:
                b.instructions = out


# rowslot(o): partition offset of o's 16 at-rows inside its actT source
# (actT1 for even o<8, shifted actT1s for odd o<8, actT2/actT2s for 8/9).
def _rowslot(o):
    if o < 8:
        return 16 * (o - (o % 2))
    return 0


def _build_program():
    import concourse.bass as bass
    import concourse.tile as tile
    from concourse import mybir

    f32 = mybir.dt.float32
    bf16 = mybir.dt.bfloat16
    AX = mybir.AxisListType
    AF = mybir.ActivationFunctionType

    nc = bass.Bass("TRN2", debug=False)

    xT_d = nc.dram_tensor("xT", [P, 2 * NPOS], bf16, kind="ExternalInput").ap()
    wpk_d = nc.dram_tensor("wpk", [2, P, OAT], bf16, kind="ExternalInput").ap()
    wsum_d = nc.dram_tensor("wsum", [2, P, OAT], bf16, kind="ExternalInput").ap()
    w2T_d = nc.dram_tensor("w2T", [P, 20 * P], bf16, kind="ExternalInput").ap()
    sel_d = nc.dram_tensor("sel", [P, 16], bf16, kind="ExternalInput").ap()
    identb_d = nc.dram_tensor("identb", [P, P], bf16, kind="ExternalInput").ap()
    out_d = nc.dram_tensor("out", [BL, OAT, HW], f32, kind="ExternalOutput").ap()

    def bcast(ap, dims, offset=0):
        return bass.AP(
            tensor=ap.tensor, offset=ap.offset + offset,
            ap=[list(ap.ap[0])] + [list(d) for d in dims],
        )

    with tile.TileContext(nc) as tc:
        with (
            tc.tile_pool(name="singles", bufs=1) as singles,
            tc.tile_pool(name="routep", bufs=3) as routep,
            tc.tile_pool(name="hp", bufs=6) as hp,
            tc.tile_pool(name="small", bufs=8) as smallp,
            tc.tile_pool(name="xfer", bufs=4) as xferp,
            tc.tile_pool(name="psum_pre", bufs=3, space="PSUM") as psP_pre,
            tc.tile_pool(name="psum_rte", bufs=2, space="PSUM") as psP_rte,
            tc.tile_pool(name="psum_g", bufs=3, space="PSUM") as psP_g,
        ):
            # x^T [(a,il), (k, pos)]: pack k covers i in [16k, 16k+16),
            # row = a*16+il (host-transposed single tile, k-major blocks)
            xT = singles.tile([P, 2 * NPOS], bf16, name="xT")
            for k in range(2):
                nc.sync.dma_start(xT[:, k * NPOS : (k + 1) * NPOS],
                                  xT_d[:, k * NPOS : (k + 1) * NPOS])
            wsum = [singles.tile([P, OAT], bf16, name=f"wsum{p}") for p in range(2)]
            for k in range(2):
                nc.sync.dma_start(wsum[k][:], wsum_d[k])
            bias01 = singles.tile([P, 1], f32, name="bias01")
            nc.vector.memset(bias01[:], BIAS_CONST)
            identb = singles.tile([P, P], bf16, name="identb")
            nc.sync.dma_start(identb[:], identb_d[:, :])
            w2T = singles.tile([P, 20 * P], bf16, name="w2T")
            nc.sync.dma_start(w2T[:], w2T_d[:, :])
            sel = singles.tile([P, 16], bf16, name="sel")
            nc.sync.dma_start(sel[:], sel_d[:, :])
            wpk = [singles.tile([P, OAT], bf16, name=f"wpk{p}") for p in range(2)]
            for p in range(2):
                nc.sync.dma_start(wpk[p][:], wpk_d[p])

            S = 2                       # max position-tiles per super-tile
            supers = [list(range(s, min(s + S, NT))) for s in range(0, NT, S)]

            # ---- prologue: all-tile iter-0 preactivations (uniform route
            # folded into wsum) + their squares (squash inputs) ----
            actT1_all = singles.tile([P, NPOS], f32, name="actT1_all")
            actT2_all = singles.tile([32, NPOS], f32, name="actT2_all")
            preb0 = singles.tile([P, NT * OAT], bf16, name="preb0")
            sqf0 = singles.tile([P, NT * OAT], bf16, name="sqf0")
            for tiles in supers:
                Sv = len(tiles)
                ps0 = psP_pre.tile([P, S * OAT], f32, tag="pre", name="ps0")
                for u, t in enumerate(tiles):
                    for k in range(2):
                        nc.tensor.matmul(
                            ps0[:, u * OAT : (u + 1) * OAT],
                            xT[:, k * NPOS + P * t : k * NPOS + P * (t + 1)],
                            wsum[k][:],
                            start=(k == 0), stop=(k == 1),
                        )
                t0 = tiles[0]
                nc.scalar.activation(
                    sqf0[:, t0 * OAT : (t0 + Sv) * OAT],
                    ps0[:, : Sv * OAT], AF.Square, bias=bias01[:],
                )
                nc.scalar.activation(
                    preb0[:, t0 * OAT : (t0 + Sv) * OAT],
                    ps0[:, : Sv * OAT], AF.Identity, bias=bias01[:],
                )

            # ---- staged routing: emit each phase across ALL supers so each
            # in-order engine stream always holds independent work from other
            # supers to fill cross-engine dependency bubbles ----
            NS = len(supers)
            st_logits = [None] * NS
            st_act = [None] * NS
            st_preb = [None] * NS
            st_poff = [0] * NS
            st_sqf = [None] * NS
            st_actT = [None] * NS
            st_hT = [None] * NS

            def emit_ph1(it, si):
                tiles = supers[si]
                Sv = len(tiles)
                e = st_logits[si]
                s = smallp.tile([P, Sv * I], f32, tag="s")
                nc.vector.reduce_sum(
                    s[:], bcast(e, [[O * I, Sv], [1, I], [I, O]]),
                    axis=AX.X,
                )
                rs = smallp.tile([P, Sv * I], bf16, tag="rs")
                with nc.allow_low_precision(
                    reason="bf16 softmax normalization within tol"
                ):
                    nc.vector.reciprocal(rs[:], s[:])
                route = smallp.tile([P, Sv * O * I], bf16, tag="route")
                nc.gpsimd.tensor_mul(
                    route.rearrange("p (u o i) -> p u o i", u=Sv, o=O),
                    e.rearrange("p (u o i) -> p u o i", u=Sv, o=O),
                    bcast(rs, [[I, Sv], [0, O], [1, I]]),
                )
                st_route[si] = route

            def emit_ph2(it, si):
                tiles = supers[si]
                Sv = len(tiles)
                route = st_route[si]
                # route_x[pos, u, (o,k), a, il] = route[pos,u,o,16k+il]
                route_x = routep.tile([P, Sv, 20, A, 16], bf16,
                                      tag="route_x")
                for u in range(Sv):
                    nc.vector.tensor_copy(
                        route_x[:, u],
                        bcast(route, [[16, 20], [0, A], [1, 16]],
                              offset=u * O * I),
                    )
                pre_ps = psP_pre.tile([P, S * OAT], f32, tag="pre",
                                      name="pre_ps")
                for u, t in enumerate(tiles):
                    # rteT[(a,il),(o,k,pos)] built in 3 PSUM chunks
                    # (8+8+4 c-blocks) so the transpose->rx handoff
                    # pipelines through a 2-deep PSUM ring
                    rx = routep.tile([P, 20 * P], bf16, tag="rx")
                    for c0, c1 in ((0, 8), (8, 16), (16, 20)):
                        nch = c1 - c0
                        rte = psP_rte.tile([P, 8 * P], bf16, tag="rte")
                        for c in range(nch):
                            nc.tensor.transpose(
                                rte[:, c * P : (c + 1) * P],
                                route_x[:, u, c0 + c],
                                identb[:],
                            )
                        nc.vector.tensor_mul(
                            rx[:, c0 * P : c1 * P].rearrange(
                                "p (o k q) -> p o k q", k=2, q=P),
                            bcast(xT, [[0, nch // 2], [NPOS, 2],
                                       [1, P]], offset=P * t),
                            rte[:, : nch * P].rearrange(
                                "p (o k q) -> p o k q", k=2, q=P),
                        )
                        # pre matmuls for this chunk (N=16 each; K=128)
                        for o in range(c0 // 2, c1 // 2):
                            for k in range(2):
                                nc.tensor.matmul(
                                    pre_ps[:, u * OAT + 16 * o :
                                           u * OAT + 16 * (o + 1)],
                                    rx[:, (2 * o + k) * P :
                                       (2 * o + k + 1) * P],
                                    wpk[k][:, 16 * o : 16 * (o + 1)],
                                    start=(k == 0), stop=(k == 1),
                                )
                # sqf first: it gates the squash chain (nsq -> scl -> act)
                sqf = smallp.tile([P, Sv * OAT], bf16, tag="sqf")
                nc.scalar.activation(
                    sqf[:], pre_ps[:, : Sv * OAT],
                    AF.Square, bias=bias01[:],
                )
                preb = smallp.tile([P, Sv * OAT], bf16, tag="preb")
                nc.scalar.activation(
                    preb[:], pre_ps[:, : Sv * OAT],
                    AF.Identity, bias=bias01[:],
                )
                st_preb[si], st_poff[si] = preb, 0
                st_sqf[si] = (sqf, 0)

            def emit_ph3(it, si):
                # squash scale + act = preb*scl; transposes of act. Non-last
                # iters also build the shifted copies for the odd-o distance
                # matmuls; the last iter writes the output copy.
                last = it == NUM_ROUTING - 1
                tiles = supers[si]
                Sv = len(tiles)
                preb, poff = st_preb[si], st_poff[si]
                sqf, soff = st_sqf[si]
                nsq = smallp.tile([P, Sv * O], f32, tag="nsq")
                nc.vector.reduce_sum(
                    nsq[:],
                    bcast(sqf, [[AT, Sv * O], [1, AT]], offset=soff),
                    axis=AX.X,
                )
                norm = smallp.tile([P, Sv * O], f32, tag="norm")
                nc.scalar.sqrt(norm[:], nsq[:])
                den = smallp.tile([P, Sv * O], f32, tag="den")
                nc.vector.tensor_scalar_add(den[:], nsq[:], 1.0)
                rden = smallp.tile([P, Sv * O], f32, tag="rden")
                nc.vector.reciprocal(rden[:], den[:])
                scl = smallp.tile([P, Sv * O], bf16, tag="scl")
                nc.vector.tensor_mul(scl[:], norm[:], rden[:])
                act = xferp.tile([P, Sv, OAT], bf16, tag="act")
                eng_act = nc.vector if last else nc.gpsimd
                eng_act.tensor_mul(
                    act.rearrange("p u (o at) -> p (u o) at", at=AT),
                    bcast(preb, [[AT, Sv * O], [1, AT]], offset=poff),
                    bcast(scl, [[1, Sv * O], [0, AT]]),
                )
                actTs = []
                for u, t in enumerate(tiles):
                    # tp col-blocks: [tp1 | tp1s | tp2 | tp2s]; the "s"
                    # variants transpose act shifted 16 columns so odd-o
                    # at-rows land 32-aligned (PE realigns partitions;
                    # vector engines cannot).
                    tp = psP_rte.tile([P, 8 * P], bf16, tag="rte")
                    nc.tensor.transpose(tp[:, 0:P], act[:, u, 0:P],
                                        identb[:])
                    if last:
                        nc.tensor.transpose(tp[0:32, 2 * P : 3 * P],
                                            act[:, u, P:OAT], identb[:])
                        nc.vector.tensor_copy(
                            actT1_all[:, P * t : P * (t + 1)], tp[:, 0:P])
                        nc.scalar.copy(actT2_all[:, P * t : P * (t + 1)],
                                       tp[0:32, 2 * P : 3 * P])
                    else:
                        nc.tensor.transpose(tp[0:P, P : 2 * P],
                                            act[:, u, 16 : 16 + P],
                                            identb[:])
                        nc.tensor.transpose(tp[0:16, 2 * P : 3 * P],
                                            act[:, u, P : P + 16],
                                            identb[:])
                        nc.tensor.transpose(tp[0:16, 3 * P : 4 * P],
                                            act[:, u, OAT - 16 : OAT],
                                            identb[:])
                        actT1c = xferp.tile([P, 2 * P], bf16,
                                            tag=f"aT1_{u}")
                        nc.scalar.copy(actT1c[:], tp[:, 0 : 2 * P])
                        actT2c = xferp.tile([16, 2 * P], bf16,
                                            tag=f"aT2_{u}")
                        nc.scalar.copy(actT2c[:], tp[0:16, 2 * P : 4 * P])
                        actTs.append((actT1c, actT2c))
                st_actT[si] = actTs
                if last and si in (2, 3):
                    # stream completed whole batches while later supers
                    # finish: tiles 0..5 cover b0..4, tiles 0..7 cover b5..6
                    b0_, b1_ = (0, 5) if si == 2 else (5, 7)
                    nc.sync.dma_start(
                        out_d.rearrange("b oat hw -> oat b hw")[
                            0:P, b0_:b1_],
                        actT1_all[:, b0_ * HW : b1_ * HW].rearrange(
                            "p (b hw) -> p b hw", hw=HW),
                    )
                    nc.sync.dma_start(
                        out_d.rearrange("b oat hw -> oat b hw")[
                            P:OAT, b0_:b1_],
                        actT2_all[:, b0_ * HW : b1_ * HW].rearrange(
                            "p (b hw) -> p b hw", hw=HW),
                    )


            # PH4a: transposed distance: gT matmuls per (o, bank), then
            # hT = xT * gT with per-chunk engine assignment:
            #   "d"  = DVE muls straight out of PSUM f32 (1x)
            #   "ad" = ACT evacs gT to SBUF bf16, DVE muls at 2x
            #   "ap" = ACT evacs gT to SBUF bf16, Pool muls
            OCHUNKS = ((0, 4), (4, 8), (8, 10))
            CHUNK_MODES = {(0, 0): "d", (0, 1): "ap", (0, 2): "ad",
                           (1, 0): "ap", (1, 1): "d", (1, 2): "ad"}

            def emit_ph4a(it, si):
                tiles = supers[si]
                actTs = st_actT[si]
                hTs = []
                for u, t in enumerate(tiles):
                    actT1c, actT2c = actTs[u]
                    hT = hp.tile([P, 2, O, P], bf16, tag="hT")
                    gT_sb = hp.tile([P, 2, O, P], bf16, tag="gT_sb")
                    for b in range(2):
                        for oc, (o0, o1) in enumerate(OCHUNKS[:2]):
                            no = o1 - o0
                            gp = psP_g.tile([P, 4 * P], f32, tag="gt")
                            for oo in range(no):
                                o = o0 + oo
                                sc = P if o % 2 else 0
                                # K=128 uniform per PSUM tile (zero-padded
                                # w2T rows) so all matmuls into one bank
                                # share a PE config
                                nc.tensor.matmul(
                                    gp[:, oo * P : (oo + 1) * P],
                                    w2T[:, (2 * o + b) * P :
                                        (2 * o + b + 1) * P],
                                    actT1c[:, sc : sc + P],
                                    start=True, stop=True,
                                )
                            mode = CHUNK_MODES[(b, oc)]
                            if mode == "d":
                                nc.vector.tensor_mul(
                                    hT[:, b, o0:o1, :],
                                    bcast(xT, [[0, no], [1, P]],
                                          offset=b * NPOS + P * t),
                                    gp[:, : no * P].rearrange(
                                        "p (o q) -> p o q", o=no),
                                )
                            else:
                                nc.scalar.copy(
                                    gT_sb[:, b, o0:o1, :],
                                    gp[:, : no * P],
                                )
                                eng = nc.gpsimd if mode == "ap" \
                                    else nc.vector
                                eng.tensor_mul(
                                    hT[:, b, o0:o1, :],
                                    bcast(xT, [[0, no], [1, P]],
                                          offset=b * NPOS + P * t),
                                    gT_sb[:, b, o0:o1, :],
                                )
                    # o=8,9 for BOTH banks in one K=16-uniform PSUM tile:
                    # one evac + one 2x mul instead of two of each
                    gp = psP_g.tile([P, 4 * P], f32, tag="gt")
                    for b in range(2):
                        for oo, o in enumerate((8, 9)):
                            sc = P if o % 2 else 0
                            nc.tensor.matmul(
                                gp[:, (2 * b + oo) * P : (2 * b + oo + 1) * P],
                                w2T[0:16, (2 * o + b) * P :
                                    (2 * o + b + 1) * P],
                                actT2c[0:16, sc : sc + P],
                                start=True, stop=True,
                            )
                    nc.scalar.copy(
                        bcast(gT_sb.rearrange("p b o q -> p (b o q)"),
                              [[O * P, 2], [P, 2], [1, P]], offset=8 * P),
                        gp[:, : 4 * P],
                    )
                    nc.vector.tensor_mul(
                        hT[:, :, 8:10, :],
                        bcast(xT, [[NPOS, 2], [0, 2], [1, P]],
                              offset=P * t),
                        gT_sb[:, :, 8:10, :],
                    )
                    hTs.append(hT)
                st_hT[si] = hTs

            def emit_ph4b(it, si):
                tiles = supers[si]
                Sv = len(tiles)
                hTs = st_hT[si]
                prev = st_logits[si]
                ed = smallp.tile([P, Sv * O * I], bf16, tag="ed")
                for u in range(Sv):
                    hT = hTs[u]
                    dist_ps = psP_pre.tile([P, S * OAT], f32, tag="pre")
                    for o in range(O):
                        for b in range(2):
                            nc.tensor.matmul(
                                dist_ps[:, 32 * o + 16 * b :
                                        32 * o + 16 * (b + 1)],
                                hT[:, b, o, :],
                                sel[:],
                                start=True, stop=True,
                            )
                    # E-state routing: keep exp(logits) instead of logits;
                    # exp evacuates the dist PSUM for free and the logits
                    # add becomes a cheap bf16 multiply
                    nc.scalar.activation(
                        ed[:, u * O * I : (u + 1) * O * I], dist_ps[:],
                        AF.Exp,
                    )
                if prev is None:
                    st_logits[si] = ed
                else:
                    e2 = smallp.tile([P, Sv * O * I], bf16, tag="e2")
                    nc.gpsimd.tensor_mul(e2[:], prev[:], ed[:])
                    st_logits[si] = e2
                emit_ph1(it + 1, si)

            st_route = [None] * NS
            for si, tiles in enumerate(supers):
                st_preb[si], st_poff[si] = preb0, tiles[0] * OAT
                st_sqf[si] = (sqf0, tiles[0] * OAT)

            # Globally skewed wavefront: stage L of super si is emitted at
            # step L + si, so every engine's in-order queue always holds
            # ready work from several (stage, super) combinations and a
            # straggling super cannot head-of-line-block the next phase.
            lanes = []
            for it in range(NUM_ROUTING):
                if it > 0:
                    lanes += [("ph2", it)]
                lanes += [("ph3", it)]
                if it < NUM_ROUTING - 1:
                    lanes += [("ph4a", it), ("ph4b", it)]
            FN = {"ph1": emit_ph1, "ph2": emit_ph2, "ph3": emit_ph3,
                  "ph4a": emit_ph4a, "ph4b": emit_ph4b}
            for step in range(len(lanes) + NS - 1):
                for L, (name, it) in enumerate(lanes):
                    si = step - L
                    if 0 <= si < NS:
                        FN[name](it, si)

            nc.sync.dma_start(
                out_d.rearrange("b oat hw -> oat b hw")[0:P, 7:BL],
                actT1_all[:, 7 * HW : NPOS].rearrange(
                    "p (b hw) -> p b hw", hw=HW),
            )
            nc.sync.dma_start(
                out_d.rearrange("b oat hw -> oat b hw")[P:OAT, 7:BL],
                actT2_all[:, 7 * HW : NPOS].rearrange(
                    "p (b hw) -> p b hw", hw=HW),
            )

    _split_multiwait_instructions(nc)
    return nc


def _get_program():
    if "nc" not in _BUILD_CACHE:
        _BUILD_CACHE["nc"] = _build_program()
    return _BUILD_CACHE["nc"]


def _host_weights(weights):
    import ml_dtypes

    w = np.asarray(weights, dtype=np.float32)        # [I, A, OAT]
    wr = w.reshape(I, A, O, AT)
    # rx-matmul weights: wpk[k][a*16+il, o*16+at] = w[16k+il, a, o, at]
    wpk = np.zeros((2, P, OAT), dtype=np.float32)
    # dense sum-over-i weights * 0.1 (uniform initial route), same row layout
    wsum = np.zeros((2, P, OAT), dtype=np.float32)
    for k in range(2):
        for il in range(16):
            for a in range(A):
                wpk[k, a * 16 + il] = wr[16 * k + il, a].reshape(OAT)
                wsum[k, a * 16 + il] = 0.1 * wr[16 * k + il, a].reshape(OAT)
    # transposed distance weights, rows zero-padded to a uniform K:
    # w2T[slotbase(o)+at, (o,b)*128 + a*16+il] = w[16b+il, a, o, at]
    # (slotbase = 16o for even o, 16(o-1) for odd o matching the shifted
    # actT copies, 0 for o=8,9; all other rows stay zero)
    w2T = np.zeros((P, 20 * P), dtype=np.float32)
    for o in range(O):
        rs_ = _rowslot(o)
        for b in range(2):
            for a in range(A):
                for il in range(16):
                    w2T[rs_ : rs_ + 16, (2 * o + b) * P + a * 16 + il] = (
                        wr[16 * b + il, a, o, :]
                    )
    # selection matrix: sel[a*16+il, il'] = (il == il')
    sel = np.zeros((P, 16), dtype=np.float32)
    for a in range(A):
        for il in range(16):
            sel[a * 16 + il, il] = 1.0
    return (
        wpk.astype(ml_dtypes.bfloat16),
        wsum.astype(ml_dtypes.bfloat16),
        w2T.astype(ml_dtypes.bfloat16),
        sel.astype(ml_dtypes.bfloat16),
    )


def kernel(x, weights, bias):
    import ml_dtypes

    assert np.allclose(np.asarray(bias, dtype=np.float32), BIAS_CONST), (
        "kernel assumes the constant 0.1 capsule bias"
    )
    x = np.ascontiguousarray(np.asarray(x, dtype=np.float32))
    identb = np.eye(P, dtype=np.float32).astype(ml_dtypes.bfloat16)
    xr = x.reshape(NCORES, BL, I, A, HW)
    # xT[c][a*16+il, k*NPOS + b*HW+hw] = x[c, b, 16k+il, a, hw]
    xT = np.ascontiguousarray(
        xr.reshape(NCORES, BL, 2, 16, A, HW)
        .transpose(0, 4, 3, 2, 1, 5)       # c, a, il, k, b, hw
        .reshape(NCORES, P, 2 * NPOS)
    ).astype(ml_dtypes.bfloat16)
    wpk, wsum, w2T, sel = _host_weights(weights)

    from concourse import bass_utils

    nc = _get_program()
    in_maps = []
    for c in range(NCORES):
        in_maps.append(
            {"xT": xT[c], "wpk": wpk, "wsum": wsum,
             "w2T": w2T, "sel": sel, "identb": identb}
        )
    res = bass_utils.run_bass_kernel_spmd(
        nc, in_maps, core_ids=list(range(NCORES))
    )
    out = np.concatenate([res.results[c]["out"] for c in range(NCORES)], axis=0)
    return out.reshape(B, O, AT, H, W).astype(np.float32)
